# revision 7
# baseline (speedup 1.0000x reference)
"""Trainium2 Bass kernel for nn_BinaryQuantumClassifier.

Math: the 4-qubit circuit collapses to a closed form. Per sample, with
theta_j = pi * (x @ W_ctq.T + b_ctq)_j  (j = 4r + i, reuse r, qubit i):
    d_i(theta) = a_i + b_i sin(theta) + c_i cos(theta)
              = a_i + R_i sin(pi * (y + b_ctq_j + phi_i/pi))
(R = hypot(b, c), phi = atan2(c, b); a/b/c derived from the fixed per-qubit
unitary RZ RY RX after RY(theta) H|0>), and the CNOT chain maps
Z-expectations to products of the d_i:
    z0 = d1 d2 d3, z1 = d0 d1, z2 = d0 d1 d2, z3 = d0 d1 d2 d3.
Output = (mean over r of z) @ W_cls.T + b_cls.

Device plan per core (8192 samples). The kernel is HBM-bound on reading x,
so x is sent as fp16 (2 B/elem, ~8.4 MB/core; fp16's 10 mantissa bits keep
the final rel err ~2e-3, well under the gate). x is relayouted on the host
so the PE uses it as the STATIONARY operand (FWL fast weight load for
16-bit), W as the tiny moving operand:
  lhsT = x-chunk [128 D x 128 samples], rhs = W-chunk [128 D x 8] fp16,
  out[128 samples, 8] accumulated over 4 D-chunks in fp32 PSUM.
8 sample-groups share one PSUM tile [128, 64] and one 1-MB x DMA; all x
DMAs are issued up-front on the sync ring (one sequential queue => data
streams continuously at the HBM roofline while PE/DVE work in its shadow);
the first two are hoisted ahead of the framework's entry barrier.
Per block (8 groups): a DVE tensor_add assembles E = psum + phase-shift
with free = j*8 + u (sample n = 128*(8b + u) + p), then the epilogue:
  k2 = ((E + 1.5*2^24) - 1.5*2^24) rounds to the nearest even integer
  (exact range reduction), r = E - k2 in [-1, 1], ScalarE Sin once,
  d = a + R sin; the CNOT products are 3 wide DVE ops on strided views
  (pairs x reuse x u), the r-mean 2 ops, and the final 4->2 linear runs
  as two parallel per-class chains (class 0 on DVE, class 1 on GpSimd)
  accumulating into two [128, 64] output tiles. Two output DMAs at the
  end, issued concurrently on the ACT and sync rings.
"""

import numpy as np

import concourse.bass as bass
import concourse.mybir as mybir
from concourse import bass_utils
from concourse.tile import TileContext

B, D, NQ = 65536, 512, 4
NCORES = 8
BC = B // NCORES            # 8192 samples per core
NCH = D // 128              # 4 K-chunks
FE = BC // 128              # 64 sample-groups per core (epilogue u index)
M2 = float(np.float32(1.5 * 2 ** 24))   # round-to-even-integer magic
PI = float(np.pi)
MM_DT = mybir.dt.float16    # PE operand dtype (x and W both fp16)
NG = BC // 128              # 64 sample-groups per core
GPB = 8                     # groups per block (per x DMA / PSUM tile)
NBLK = NG // GPB            # 8 blocks
JB = 8 * GPB                # 64: width of a block's epilogue tile (j*8+u)
GL = GPB * NCH * 128        # 4096: free width of one x block (fp16)
KPERM = (2, 0, 3, 1)        # z_k order in the Mn tile (z2, z0, z3, z1)
AL = mybir.AluOpType
AF = mybir.ActivationFunctionType
F32 = mybir.dt.float32


def _split_waits(nc, max_waits=1):
    """walrus in this env accepts at most one sync-wait per instruction;
    move extras onto preceding same-engine NoOps."""
    for fn in nc.m.functions:
        for blk in fn.blocks:
            new_list = []
            for inst in blk.instructions:
                si = inst.sync_info
                if si is not None and len(si.on_wait) > max_waits:
                    waits = list(si.on_wait)
                    keep, extra = waits[-max_waits:], waits[:-max_waits]
                    for k, w in enumerate(extra):
                        new_list.append(mybir.InstNoOp(
                            name=f"{inst.name}-ws{k}", engine=inst.engine,
                            ins=[], outs=[],
                            sync_info=mybir.SyncInfo(on_wait=[w], on_update=[])))
                    si.on_wait = keep
                    inst.sync_info = si
                new_list.append(inst)
            blk.instructions = new_list


def _hoist_dmas(nc, n_sync=2, n_act=2):
    """Move the first wait-free DMA issues of the sync (x tiles) and ACT
    (constants) rings from the tile block into `main`, ahead of the
    all-engine entry barrier, so the x stream starts ~1 us earlier."""
    blks = [b for f in nc.m.functions for b in f.blocks]
    main = next(b for b in blks if b.name == "main")
    tile = max(blks, key=lambda b: len(b.instructions))
    want = {mybir.EngineType.SP: n_sync, mybir.EngineType.Activation: n_act}
    hoisted, rest = [], []
    for inst in tile.instructions:
        if (want.get(inst.engine, 0) > 0 and isinstance(inst, mybir.InstDMACopy)
                and not (inst.sync_info and inst.sync_info.on_wait)):
            hoisted.append(inst)
            want[inst.engine] -= 1
        else:
            rest.append(inst)
    tile.instructions = rest
    # insert before the entry barrier (first InstDrain in main)
    idx = next(i for i, inst in enumerate(main.instructions)
               if isinstance(inst, mybir.InstDrain))
    main.instructions = (main.instructions[:idx] + hoisted
                         + main.instructions[idx:])


def _build_nc():
    nc = bass.Bass("TRN2", target_bir_lowering=False)
    # x relayout (fp16): xa[p, m*512 + k*128 + ms] = x[128m + ms, 128k + p]
    xa_d = nc.dram_tensor("xa", [128, BC * NCH], MM_DT, kind="ExternalInput").ap()
    # W chunks: [k*8 + j] = whi chunk k; [32 + k*8 + j] = wlo chunk k
    wcat_d = nc.dram_tensor("wcat", [128, 64], MM_DT, kind="ExternalInput").ap()
    # cvf: [0:16]=cv (W_cls/b_cls), then bsT | RT | aT each [128, JB] (j*8+u),
    # then CW1 [128, 4*GPB] (class-1 weights in KPERM t-order, replicated
    # along u) and CB1 [128, GPB] (class-1 bias) for the GpSimd chain
    CVW = 16 + 3 * JB
    cvf_d = nc.dram_tensor("cvf", [128, CVW + 5 * GPB], F32, kind="ExternalInput").ap()
    # o[p, c*FE + 8*b + u] = out_c(sample 128*(8b + u) + p)
    o_d = nc.dram_tensor("o", [128, 2 * FE], F32, kind="ExternalOutput").ap()

    with TileContext(nc) as tc:
        with tc.tile_pool(name="wp", bufs=1) as wpool, \
             tc.tile_pool(name="xp", bufs=NBLK) as xpool, \
             tc.tile_pool(name="pp", bufs=4, space="PSUM") as pspool, \
             tc.tile_pool(name="ep", bufs=1) as epool:
            # constants on the ACT ring (parallel with x on the sync ring)
            wsb = wpool.tile([128, 64], MM_DT)
            nc.scalar.dma_start(wsb[:], wcat_d[:])
            cvsb = wpool.tile([128, CVW + 5 * GPB], F32)
            nc.scalar.dma_start(cvsb[:], cvf_d[:])
            cv = cvsb[:, 0:16]
            bsT = cvsb[:, 16:16 + JB]
            RT = cvsb[:, 16 + JB:16 + 2 * JB]
            aT = cvsb[:, 16 + 2 * JB:16 + 3 * JB]
            CW1 = cvsb[:, CVW:CVW + 4 * GPB]
            CB1 = cvsb[:, CVW + 4 * GPB:CVW + 5 * GPB]
            bs3 = bsT.rearrange("p (j u) -> p j u", j=8)

            # all x DMAs up-front, one sequential queue
            Las = []
            for g in range(NBLK):
                La = xpool.tile([128, GL], MM_DT, tag="La", name=f"La{g}")
                nc.sync.dma_start(La[:], xa_d[:, g * GL:(g + 1) * GL])
                Las.append(La)

            Oc0 = epool.tile([128, FE], F32, name="Oc0")
            Oc1 = epool.tile([128, FE], F32, name="Oc1")
            for b in range(NBLK):
                La = Las[b]
                ps = pspool.tile([128, GPB * 8], F32, tag="ps", name=f"ps{b}")
                for mm in range(GPB):
                    for k in range(NCH):
                        off = mm * (NCH * 128) + k * 128
                        out_sl = ps[:, 8 * mm:8 * mm + 8]
                        nc.tensor.matmul(out_sl, La[:, off:off + 128],
                                         wsb[:, 8 * k:8 * k + 8],
                                         start=(k == 0), stop=(k == NCH - 1))

                # ---- epilogue for this block (j*8 + u layout) ----
                E = epool.tile([128, JB], F32, name=f"E{b}")
                e3 = E.rearrange("p (j u) -> p j u", j=8)
                # E[:, 8j + m] = ps[:, 8m + j] + bs (phase shift)
                nc.vector.tensor_add(e3[:], ps.rearrange("p (m j) -> p j m", j=8),
                                     bs3[:])
                k2 = epool.tile([128, JB], F32, name=f"k2_{b}")
                r_ = epool.tile([128, JB], F32, name=f"r_{b}")
                s_ = epool.tile([128, JB], F32, name=f"s_{b}")
                t1 = epool.tile([128, JB], F32, name=f"t1_{b}")
                d_ = epool.tile([128, JB], F32, name=f"d_{b}")
                nc.vector.tensor_scalar(k2[:], E[:], M2, M2, AL.add, AL.subtract)
                nc.vector.tensor_sub(r_[:], E[:], k2[:])       # E mod 2 -> [-1, 1]
                nc.scalar.activation(s_[:], r_[:], AF.Sin, scale=PI)
                nc.vector.tensor_mul(t1[:], s_[:], RT)         # R sin
                nc.vector.tensor_add(d_[:], t1[:], aT)         # d = a + R sin

                # CNOT products, 3 wide ops on strided views.
                # d4[i, r, u] = d_(j = 4r + i); PQT slots s: z1, e=d1d2, z2, z0, z3
                d4 = d_.rearrange("p (r i u) -> p i r u", r=2, i=4)
                PQT = epool.tile([128, 5 * 2 * GPB], F32, name=f"PQT{b}")
                P5 = PQT.rearrange("p (s r u) -> p s r u", s=5, r=2)
                nc.vector.tensor_mul(P5[:, 0:2], d4[:, 0:2], d4[:, 1:3])  # z1, e
                nc.vector.tensor_mul(P5[:, 2:4], P5[:, 0:2], d4[:, 2:4])  # z2, z0
                nc.vector.tensor_mul(P5[:, 4:5], P5[:, 2:3], d4[:, 3:4])  # z3
                # mean over r (x2 folded into cv): Mn blocks in KPERM order
                Mn = epool.tile([128, 4 * GPB], F32, name=f"Mn{b}")
                M4 = Mn.rearrange("p (t r u) -> p t r u", t=4, r=1)
                nc.vector.tensor_add(M4[:, 0:3], P5[:, 2:5, 0:1], P5[:, 2:5, 1:2])
                nc.vector.tensor_add(M4[:, 3:4], P5[:, 0:1, 0:1], P5[:, 0:1, 1:2])

                def mk(t):
                    return Mn[:, t * GPB:(t + 1) * GPB]

                # final linear (W' = 0.5*W_cls, KPERM column order in cv):
                # class 0 on DVE (scalar-ptr chain), class 1 on GpSimd
                # (plain tensor_tensor with broadcast const tiles) — parallel
                c = 0
                oa = epool.tile([128, GPB], F32, name=f"oa_{b}")
                ob = epool.tile([128, GPB], F32, name=f"ob_{b}")
                nc.vector.tensor_scalar(oa[:], mk(0), cv[:, 4:5],
                                        cv[:, 12:13], AL.mult, AL.add)
                nc.vector.scalar_tensor_tensor(ob[:], mk(1), cv[:, 5:6],
                                               oa[:], AL.mult, AL.add)
                nc.vector.scalar_tensor_tensor(oa[:], mk(2), cv[:, 6:7],
                                               ob[:], AL.mult, AL.add)
                nc.vector.scalar_tensor_tensor(Oc0[:, GPB * b:GPB * (b + 1)],
                                               mk(3), cv[:, 7:8],
                                               oa[:], AL.mult, AL.add)
                P_ = epool.tile([128, 4 * GPB], F32, name=f"P_{b}")
                A_ = epool.tile([128, 2 * GPB], F32, name=f"A_{b}")
                nc.gpsimd.tensor_mul(P_[:], Mn[:], CW1[:])
                nc.gpsimd.tensor_add(A_[:], P_[:, 0:2 * GPB], P_[:, 2 * GPB:4 * GPB])
                nc.gpsimd.tensor_add(A_[:, 0:GPB], A_[:, 0:GPB], A_[:, GPB:2 * GPB])
                nc.gpsimd.tensor_add(Oc1[:, GPB * b:GPB * (b + 1)],
                                     A_[:, 0:GPB], CB1[:])
            # two output DMAs on parallel rings (ACT + sync)
            nc.scalar.dma_start(o_d[:, 0:FE], Oc0[:])
            nc.sync.dma_start(o_d[:, FE:2 * FE], Oc1[:])

    return nc


_NC_CACHE = {}


def _get_nc(split=True):
    key = ("nc", split)
    if key not in _NC_CACHE:
        nc = _build_nc()
        _hoist_dmas(nc)
        if split:
            _split_waits(nc)
        _NC_CACHE[key] = nc
    return _NC_CACHE[key]


def _qubit_abc(q_params):
    """Exact (a_i, b_i, c_i) with d_i(theta) = a + b sin(theta) + c cos(theta)."""
    out = np.zeros((NQ, 3), np.float64)
    for i in range(NQ):
        pa, pb, pc = [float(v) for v in q_params[3 * i:3 * i + 3]]

        def rx(t):
            return np.array([[np.cos(t / 2), -1j * np.sin(t / 2)],
                             [-1j * np.sin(t / 2), np.cos(t / 2)]])

        def ry(t):
            return np.array([[np.cos(t / 2), -np.sin(t / 2)],
                             [np.sin(t / 2), np.cos(t / 2)]])

        def rz(t):
            return np.array([[np.exp(-0.5j * t), 0], [0, np.exp(0.5j * t)]])

        H = np.array([[1, 1], [1, -1]]) / np.sqrt(2)
        U = rz(pc) @ ry(pb) @ rx(pa)

        def dfun(theta):
            v = U @ ry(theta) @ H @ np.array([1.0, 0.0])
            pr = np.abs(v) ** 2
            return pr[0] - pr[1]

        d0, dpi, dh = dfun(0.0), dfun(np.pi), dfun(np.pi / 2)
        a = (d0 + dpi) / 2
        c = (d0 - dpi) / 2
        b = dh - a
        out[i] = (a, b, c)
    return out


def _make_consts(b_ctq, q_params, W_cls, b_cls):
    abc = _qubit_abc(q_params)
    CVW = 16 + 3 * JB
    cvf = np.zeros((128, CVW + 5 * GPB), np.float32)
    wp = 0.5 * np.asarray(W_cls, np.float64)      # mean over r folded in
    for c in range(2):
        for t in range(4):
            cvf[:, 4 + 4 * c + t] = np.float32(wp[c, KPERM[t]])
        cvf[:, 12 + c] = np.float32(b_cls[c])
    for j in range(8):
        i = j % 4
        a, b, c_ = abc[i]
        R = np.hypot(b, c_)
        phi = np.arctan2(c_, b)
        cvf[:, 16 + j * GPB:16 + (j + 1) * GPB] = np.float32(b_ctq[j] + phi / np.pi)
        cvf[:, 16 + JB + j * GPB:16 + JB + (j + 1) * GPB] = np.float32(R)
        cvf[:, 16 + 2 * JB + j * GPB:16 + 2 * JB + (j + 1) * GPB] = np.float32(a)
    for t in range(4):
        cvf[:, CVW + t * GPB:CVW + (t + 1) * GPB] = np.float32(wp[1, KPERM[t]])
    cvf[:, CVW + 4 * GPB:CVW + 5 * GPB] = np.float32(b_cls[1])
    return cvf


def make_in_maps(x, W_ctq, b_ctq, q_params, W_cls, b_cls):
    f16 = np.float16
    wt = np.asarray(W_ctq, np.float32).T                        # [512, 8]
    whi = wt.astype(f16)
    wlo = (wt - whi.astype(np.float32)).astype(f16)
    wcat = np.zeros((128, 64), f16)
    for k in range(NCH):
        wcat[:, 8 * k:8 * (k + 1)] = whi[128 * k:128 * (k + 1), :]
        wcat[:, 32 + 8 * k:32 + 8 * (k + 1)] = wlo[128 * k:128 * (k + 1), :]
    cvf = _make_consts(np.asarray(b_ctq, np.float32),
                       np.asarray(q_params, np.float32),
                       np.asarray(W_cls, np.float32),
                       np.asarray(b_cls, np.float32))
    x = np.asarray(x, np.float32)
    in_maps = []
    for c in range(NCORES):
        xs = x[c * BC:(c + 1) * BC]                             # [8192, 512]
        # relayout: [p, m*512 + k*128 + ms] = xs[128 m + ms, 128 k + p]
        xa = np.ascontiguousarray(
            xs.reshape(NG, 128, NCH, 128).transpose(3, 0, 2, 1)
            .reshape(128, BC * NCH).astype(f16))
        in_maps.append({"xa": xa, "wcat": wcat, "cvf": cvf})
    return in_maps


def assemble_output(results):
    out = np.empty((B, 2), np.float32)
    for core in range(NCORES):
        o = results[core]["o"]                                   # [128, 2*FE]
        # o[p, c*FE + u] = out_c(sample 128*u + p)
        out[core * BC:(core + 1) * BC] = (
            o.reshape(128, 2, FE).transpose(2, 0, 1).reshape(BC, 2))
    return out


def kernel(x, W_ctq, b_ctq, q_params, W_cls, b_cls):
    nc = _get_nc()
    in_maps = make_in_maps(x, W_ctq, b_ctq, q_params, W_cls, b_cls)
    res = bass_utils.run_bass_kernel_spmd(nc, in_maps, core_ids=list(range(NCORES)))
    return assemble_output(res.results)


# revision 9
# speedup vs baseline: 1.3086x; 1.3086x over previous
"""Trainium2 Bass kernel for nn_BinaryQuantumClassifier.

Math: the 4-qubit circuit collapses to a closed form. Per sample, with
theta_j = pi * (x @ W_ctq.T + b_ctq)_j  (j = 4r + i, reuse r, qubit i):
    d_i(theta) = a_i + b_i sin(theta) + c_i cos(theta)
              = a_i + R_i sin(pi * (y + b_ctq_j + phi_i/pi))
(R = hypot(b, c), phi = atan2(c, b); a/b/c derived from the fixed per-qubit
unitary RZ RY RX after RY(theta) H|0>), and the CNOT chain maps
Z-expectations to products of the d_i:
    z0 = d1 d2 d3, z1 = d0 d1, z2 = d0 d1 d2, z3 = d0 d1 d2 d3.
Output = (mean over r of z) @ W_cls.T + b_cls.
The R factors are divided out of d (d' = s + a/R) and folded into the
final linear weights, so the epilogue per value is one add after the sin.

Device plan per core (8192 samples). The kernel is HBM-bound on reading x,
so x is sent as fp16 (2 B/elem, ~8.4 MB/core; fp16's 10 mantissa bits keep
the final rel err ~2.5e-3, well under the gate). x is relayouted on the
host so the PE uses it as the STATIONARY operand (FWL fast weight load),
W as the tiny moving operand:
  lhsT = x-chunk [128 D x 128 samples], rhs = W-chunk [128 D x 8] fp16,
  out[128 samples, 8] accumulated over 4 D-chunks in fp32 PSUM.
x arrives as 10 DMAs (6 x 1 MB + 4 x 512 KB at the end, so the final
matmul burst after the last byte is short), all issued up-front on the
sync ring (one sequential queue => data streams continuously at the HBM
roofline); the first two are hoisted ahead of the framework entry barrier.
Constants ride the ACT ring.
Epilogue per QUARTER (16 groups, one [128, 128] PSUM tile, so DVE ops are
wide and few; fp16 intermediates get the DVE 2x/4x perf modes):
  E = psum + phase-shift (fp32, free = j*16 + u), k2 = ((E + 1.5*2^24) -
  1.5*2^24) rounds to the nearest even integer (exact range reduction),
  r = E - k2 in [-1, 1] (fp16), ScalarE Sin, d' = s + a/R, CNOT products
  as 3 wide muls on strided views (pair x reuse x u), r-mean 2 ops, final
  4->2 linear as two 4-op scalar chains; one [128, 32] fp16 output tile
  per quarter, DMA'd on the ACT ring as soon as it is ready.
"""

import numpy as np

import concourse.bass as bass
import concourse.mybir as mybir
from concourse import bass_utils
from concourse.tile import TileContext

B, D, NQ = 65536, 512, 4
NCORES = 8
BC = B // NCORES            # 8192 samples per core
NCH = D // 128              # 4 K-chunks
FE = BC // 128              # 64 sample-groups per core (epilogue u index)
M2 = float(np.float32(1.5 * 2 ** 24))   # round-to-even-integer magic
PI = float(np.pi)
MM_DT = mybir.dt.float16    # PE operand dtype (x and W both fp16)
EP_DT = mybir.dt.float16    # epilogue intermediate dtype (DVE 2x/4x modes)
NG = BC // 128              # 64 sample-groups per core
NQT = 4                     # epilogue quarters
FQ = 16                     # groups per quarter
JQ = 8 * FQ                 # 128: width of a quarter's epilogue tile (j*16+u)
TGRP = [8, 8, 8, 8, 8, 8, 4, 4, 4, 4]   # groups per x DMA tile
KPERM = (2, 0, 3, 1)        # z_k order in the Mn tile (z2, z0, z3, z1)
AL = mybir.AluOpType
AF = mybir.ActivationFunctionType
F32 = mybir.dt.float32


def _split_waits(nc, max_waits=1):
    """walrus in this env accepts at most one sync-wait per instruction;
    move extras onto preceding same-engine NoOps."""
    for fn in nc.m.functions:
        for blk in fn.blocks:
            new_list = []
            for inst in blk.instructions:
                si = inst.sync_info
                if si is not None and len(si.on_wait) > max_waits:
                    waits = list(si.on_wait)
                    keep, extra = waits[-max_waits:], waits[:-max_waits]
                    for k, w in enumerate(extra):
                        new_list.append(mybir.InstNoOp(
                            name=f"{inst.name}-ws{k}", engine=inst.engine,
                            ins=[], outs=[],
                            sync_info=mybir.SyncInfo(on_wait=[w], on_update=[])))
                    si.on_wait = keep
                    inst.sync_info = si
                new_list.append(inst)
            blk.instructions = new_list


def _hoist_dmas(nc, n_sync=2, n_act=3):
    """Move the first wait-free DMA issues of the sync (x tiles) and ACT
    (constants) rings from the tile block into `main`, ahead of the
    all-engine entry barrier, so the x stream starts ~1 us earlier."""
    blks = [b for f in nc.m.functions for b in f.blocks]
    main = next(b for b in blks if b.name == "main")
    tile = max(blks, key=lambda b: len(b.instructions))
    want = {mybir.EngineType.SP: n_sync, mybir.EngineType.Activation: n_act}
    hoisted, rest = [], []
    for inst in tile.instructions:
        if (want.get(inst.engine, 0) > 0 and isinstance(inst, mybir.InstDMACopy)
                and not (inst.sync_info and inst.sync_info.on_wait)):
            hoisted.append(inst)
            want[inst.engine] -= 1
        else:
            rest.append(inst)
    tile.instructions = rest
    # insert before the entry barrier (first InstDrain in main)
    idx = next(i for i, inst in enumerate(main.instructions)
               if isinstance(inst, mybir.InstDrain))
    main.instructions = (main.instructions[:idx] + hoisted
                         + main.instructions[idx:])


def _build_nc():
    nc = bass.Bass("TRN2", target_bir_lowering=False)
    # x relayout (fp16): xa[p, m*512 + k*128 + ms] = x[128m + ms, 128k + p]
    xa_d = nc.dram_tensor("xa", [128, BC * NCH], MM_DT, kind="ExternalInput").ap()
    # W chunks: [k*8 + j] = W.T chunk k (fp16)
    wcat_d = nc.dram_tensor("wcat", [128, 32], MM_DT, kind="ExternalInput").ap()
    # fp32 consts: bsT [128, JQ] (phase shift, j*16+u) + cv [128, 16]
    cf32_d = nc.dram_tensor("cf32", [128, JQ + 16], F32, kind="ExternalInput").ap()
    # fp16 consts: AoR [128, JQ]
    cf16_d = nc.dram_tensor("cf16", [128, JQ], EP_DT, kind="ExternalInput").ap()
    # o[p, 32*qi + 16*c + uq] = out_c(sample 128*(16*qi + uq) + p), fp16
    o_d = nc.dram_tensor("o", [128, 2 * FE], EP_DT, kind="ExternalOutput").ap()

    tile_g0 = np.cumsum([0] + TGRP)     # first group of each x tile

    with TileContext(nc) as tc:
        with tc.tile_pool(name="wp", bufs=1) as wpool, \
             tc.tile_pool(name="xp", bufs=len(TGRP)) as xpool, \
             tc.tile_pool(name="pp", bufs=4, space="PSUM") as pspool, \
             tc.tile_pool(name="ep", bufs=1) as epool:
            # constants on the ACT ring (parallel with x on the sync ring)
            wsb = wpool.tile([128, 32], MM_DT)
            nc.scalar.dma_start(wsb[:], wcat_d[:])
            c32 = wpool.tile([128, JQ + 16], F32)
            nc.scalar.dma_start(c32[:], cf32_d[:])
            c16 = wpool.tile([128, JQ], EP_DT)
            nc.scalar.dma_start(c16[:], cf16_d[:])
            bs3 = c32[:, 0:JQ].rearrange("p (j u) -> p j u", j=8)
            cv = c32[:, JQ:JQ + 16]
            AoR = c16[:, 0:JQ]

            # all x DMAs up-front, one sequential queue
            Las = []
            for t, gw in enumerate(TGRP):
                gl = gw * NCH * 128
                La = xpool.tile([128, gl], MM_DT, tag="La", name=f"La{t}")
                off = int(tile_g0[t]) * NCH * 128
                nc.sync.dma_start(La[:], xa_d[:, off:off + gl])
                Las.append(La)

            for qi in range(NQT):
                psq = pspool.tile([128, JQ], F32, tag="ps", name=f"ps{qi}")
                for t, gw in enumerate(TGRP):
                    for lg in range(gw):
                        g = int(tile_g0[t]) + lg
                        if not (FQ * qi <= g < FQ * (qi + 1)):
                            continue
                        mq = g - FQ * qi
                        for k in range(NCH):
                            off = lg * (NCH * 128) + k * 128
                            nc.tensor.matmul(psq[:, 8 * mq:8 * mq + 8],
                                             Las[t][:, off:off + 128],
                                             wsb[:, 8 * k:8 * k + 8],
                                             start=(k == 0), stop=(k == NCH - 1))

                # ---- epilogue for this quarter (j*16 + u layout) ----
                E = epool.tile([128, JQ], F32, name=f"E{qi}")
                e3 = E.rearrange("p (j u) -> p j u", j=8)
                # E[:, 16j + m] = ps[:, 8m + j] + bs (phase shift)
                nc.vector.tensor_add(e3[:], psq.rearrange("p (m j) -> p j m", j=8),
                                     bs3[:])
                k2 = epool.tile([128, JQ], F32, name=f"k2_{qi}")
                r_ = epool.tile([128, JQ], EP_DT, name=f"r_{qi}")
                s_ = epool.tile([128, JQ], EP_DT, name=f"s_{qi}")
                d_ = epool.tile([128, JQ], EP_DT, name=f"d_{qi}")
                nc.vector.tensor_scalar(k2[:], E[:], M2, M2, AL.add, AL.subtract)
                nc.vector.tensor_sub(r_[:], E[:], k2[:])       # E mod 2 -> [-1, 1]
                nc.scalar.activation(s_[:], r_[:], AF.Sin, scale=PI)
                nc.vector.tensor_add(d_[:], s_[:], AoR)        # d' = sin + a/R

                # CNOT products, 3 wide muls on strided views.
                # d4[i, r, u] = d'(j = 4r + i); PQT slots s: z1, e=d1d2, z2, z0, z3
                d4 = d_.rearrange("p (r i u) -> p i r u", r=2, i=4)
                PQT = epool.tile([128, 5 * 2 * FQ], EP_DT, name=f"PQT{qi}")
                P5 = PQT.rearrange("p (s r u) -> p s r u", s=5, r=2)
                nc.vector.tensor_mul(P5[:, 0:2], d4[:, 0:2], d4[:, 1:3])  # z1, e
                nc.vector.tensor_mul(P5[:, 2:4], P5[:, 0:2], d4[:, 2:4])  # z2, z0
                nc.vector.tensor_mul(P5[:, 4:5], P5[:, 2:3], d4[:, 3:4])  # z3
                # mean over r (x2 folded into cv): Mn blocks in KPERM order
                Mn = epool.tile([128, 4 * FQ], EP_DT, name=f"Mn{qi}")
                M4 = Mn.rearrange("p (t r u) -> p t r u", t=4, r=1)
                nc.vector.tensor_add(M4[:, 0:3], P5[:, 2:5, 0:1], P5[:, 2:5, 1:2])
                nc.vector.tensor_add(M4[:, 3:4], P5[:, 0:1, 0:1], P5[:, 0:1, 1:2])

                def mk(t):
                    return Mn[:, t * FQ:(t + 1) * FQ]

                # final linear (W' = 0.5*R-prod*W_cls, KPERM order in cv)
                Oq = epool.tile([128, 2 * FQ], EP_DT, name=f"Oq{qi}")
                oa = epool.tile([128, FQ], EP_DT, name=f"oa{qi}")
                ob = epool.tile([128, FQ], EP_DT, name=f"ob{qi}")
                for c in range(2):
                    oq = Oq[:, c * FQ:(c + 1) * FQ]
                    nc.vector.tensor_scalar(oa[:], mk(0), cv[:, 4 + 4 * c:5 + 4 * c],
                                            cv[:, 12 + c:13 + c], AL.mult, AL.add)
                    nc.vector.scalar_tensor_tensor(ob[:], mk(1),
                                                   cv[:, 5 + 4 * c:6 + 4 * c],
                                                   oa[:], AL.mult, AL.add)
                    nc.vector.scalar_tensor_tensor(oa[:], mk(2),
                                                   cv[:, 6 + 4 * c:7 + 4 * c],
                                                   ob[:], AL.mult, AL.add)
                    nc.vector.scalar_tensor_tensor(oq, mk(3),
                                                   cv[:, 7 + 4 * c:8 + 4 * c],
                                                   oa[:], AL.mult, AL.add)
                nc.scalar.dma_start(o_d[:, 2 * FQ * qi:2 * FQ * (qi + 1)], Oq[:])

    return nc


_NC_CACHE = {}


def _get_nc(split=True):
    key = ("nc", split)
    if key not in _NC_CACHE:
        nc = _build_nc()
        _hoist_dmas(nc)
        if split:
            _split_waits(nc)
        _NC_CACHE[key] = nc
    return _NC_CACHE[key]


def _qubit_abc(q_params):
    """Exact (a_i, b_i, c_i) with d_i(theta) = a + b sin(theta) + c cos(theta)."""
    out = np.zeros((NQ, 3), np.float64)
    for i in range(NQ):
        pa, pb, pc = [float(v) for v in q_params[3 * i:3 * i + 3]]

        def rx(t):
            return np.array([[np.cos(t / 2), -1j * np.sin(t / 2)],
                             [-1j * np.sin(t / 2), np.cos(t / 2)]])

        def ry(t):
            return np.array([[np.cos(t / 2), -np.sin(t / 2)],
                             [np.sin(t / 2), np.cos(t / 2)]])

        def rz(t):
            return np.array([[np.exp(-0.5j * t), 0], [0, np.exp(0.5j * t)]])

        H = np.array([[1, 1], [1, -1]]) / np.sqrt(2)
        U = rz(pc) @ ry(pb) @ rx(pa)

        def dfun(theta):
            v = U @ ry(theta) @ H @ np.array([1.0, 0.0])
            pr = np.abs(v) ** 2
            return pr[0] - pr[1]

        d0, dpi, dh = dfun(0.0), dfun(np.pi), dfun(np.pi / 2)
        a = (d0 + dpi) / 2
        c = (d0 - dpi) / 2
        b = dh - a
        out[i] = (a, b, c)
    return out


def _make_consts(b_ctq, q_params, W_cls, b_cls):
    abc = _qubit_abc(q_params)
    Rq = np.maximum(np.hypot(abc[:, 1], abc[:, 2]), 1e-3)       # [4]
    gam = np.array([Rq[1] * Rq[2] * Rq[3], Rq[0] * Rq[1],
                    Rq[0] * Rq[1] * Rq[2], Rq[0] * Rq[1] * Rq[2] * Rq[3]])
    cf32 = np.zeros((128, JQ + 16), np.float32)
    cf16 = np.zeros((128, JQ), np.float16)
    for j in range(8):
        i = j % 4
        a, b, c_ = abc[i]
        phi = np.arctan2(c_, b)
        cf32[:, j * FQ:(j + 1) * FQ] = np.float32(b_ctq[j] + phi / np.pi)
        cf16[:, j * FQ:(j + 1) * FQ] = np.float16(a / Rq[i])
    wp = 0.5 * np.asarray(W_cls, np.float64) * gam[None, :]     # mean + R folded
    for c in range(2):
        for t in range(4):
            cf32[:, JQ + 4 + 4 * c + t] = np.float32(wp[c, KPERM[t]])
        cf32[:, JQ + 12 + c] = np.float32(b_cls[c])
    return cf32, cf16


def make_in_maps(x, W_ctq, b_ctq, q_params, W_cls, b_cls):
    f16 = np.float16
    wt = np.asarray(W_ctq, np.float32).T                        # [512, 8]
    wcat = np.zeros((128, 32), f16)
    for k in range(NCH):
        wcat[:, 8 * k:8 * (k + 1)] = wt[128 * k:128 * (k + 1), :].astype(f16)
    cf32, cf16 = _make_consts(np.asarray(b_ctq, np.float32),
                              np.asarray(q_params, np.float32),
                              np.asarray(W_cls, np.float32),
                              np.asarray(b_cls, np.float32))
    x = np.asarray(x, np.float32)
    in_maps = []
    for c in range(NCORES):
        xs = x[c * BC:(c + 1) * BC]                             # [8192, 512]
        # relayout: [p, m*512 + k*128 + ms] = xs[128 m + ms, 128 k + p]
        xa = np.ascontiguousarray(
            xs.reshape(NG, 128, NCH, 128).transpose(3, 0, 2, 1)
            .reshape(128, BC * NCH).astype(f16))
        in_maps.append({"xa": xa, "wcat": wcat, "cf32": cf32, "cf16": cf16})
    return in_maps


def assemble_output(results):
    out = np.empty((B, 2), np.float32)
    for core in range(NCORES):
        o = np.asarray(results[core]["o"], np.float32)           # [128, 2*FE]
        # o[p, 32*qi + 16*c + uq] = out_c(sample 128*(16*qi + uq) + p)
        out[core * BC:(core + 1) * BC] = (
            o.reshape(128, NQT, 2, FQ).transpose(1, 3, 0, 2).reshape(BC, 2))
    return out


def kernel(x, W_ctq, b_ctq, q_params, W_cls, b_cls):
    nc = _get_nc()
    in_maps = make_in_maps(x, W_ctq, b_ctq, q_params, W_cls, b_cls)
    res = bass_utils.run_bass_kernel_spmd(nc, in_maps, core_ids=list(range(NCORES)))
    return assemble_output(res.results)
